# revision 16
# baseline (speedup 1.0000x reference)
"""CoAtNet relative attention kernel for Trainium2 (Bass/Tile), 8 NeuronCores.

Problem (per full input):
  x [16, 256, 32, 32] f32; Wq/Wk/Wv [256, 256]; Wo [256, 256]; bo [256];
  rel_bias [8, 3969]; rel_idx [1024, 1024] int32 (static pattern).
  out[b] = softmax(q k^T / sqrt(d) + bias) v  projected back, heads=8, d=32.

Sharding: data-parallel over batch — each of the 8 cores handles 2 batches
with identical programs (SPMD, no collectives).

Key structural facts used:
  * rel_idx[p, q] == (q - p) + 1056 exactly (the reference's quirky *W stride
    collapses the 2D relative index to 1D Toeplitz).  So the [1024, 1024]
    bias matrix per head is bias[p, q] = rel_bias[h, q - p + 1056] and any
    [128, width] tile of it (keys on partitions) is a contiguous slice of a
    small "sheared" tile  G[h, i, j'] = rel_bias[h, 1952 + i - j']  of shape
    [128, 1920].  No gather on device at all.  The bias is applied as
    exp(S+B) = exp(S) * exp(B) with exp(B) precomputed, so the application
    is a bf16 2x-mode multiply instead of an fp32 1x add.
  * Everything is computed in "transposed" layout so no transposes are ever
    needed: x arrives as [c, n] per batch; Q^T/K^T = W @ x are [d_all, n];
    scores are built as S^T [keys, queries]; P@V uses lhsT = V directly;
    and the final projection produces out^T [c, n], exactly the output
    memory layout.
  * The kernel is ACT(exp)-throughput-bound: 16.8M exps/core at 1 elem/
    lane/cycle @1.2GHz is a ~110us floor.  The schedule is built so the
    ScalarE never waits: each strip's exp is SPLIT into two ACTIVATEs over
    separate PSUM tiles (st_lo banks 0-1, st_hi banks 2-3) so the next
    strip's score matmuls can overwrite the low banks while ACT still
    processes the high banks (the single-buffered [128,2048] fp32 score
    tile cannot be double-buffered: TRN2 matmuls write fp32-only and PSUM
    has just 8 banks).  PV/den matmuls run TWO strips behind the score
    matmuls so a DVE-gated PV never head-of-line blocks ready ST work in
    the PE FIFO.  All projection / output-projection / softmax-
    normalization work is dripped into per-strip slack slots on the other
    engines.
"""

import numpy as np
from collections import deque
from contextlib import ExitStack

import concourse.bass as bass
import concourse.bacc as bacc
import concourse.mybir as mybir
import concourse.tile as tile
from concourse import bass_utils
from concourse._compat import with_exitstack

HEADS = 8
D = 32  # head dim
C = 256  # channels = heads * D
N = 1024  # tokens = 32 * 32
B_LOC = 2  # batches per core
N_CORES = 8
SCALE = D ** -0.5
GW = 1920  # sheared bias tile width
G0 = 1952  # G[h, i, j'] = rel_bias[h, G0 + i - j']

F32 = mybir.dt.float32
BF16 = mybir.dt.bfloat16
AF = mybir.ActivationFunctionType


@with_exitstack
def _emit(ctx: ExitStack, tc: tile.TileContext, io: dict):
    nc = tc.nc
    x, wqT, wkT, wvT, woT, bo, eb, out = (
        io[k] for k in ("x", "wqT", "wkT", "wvT", "woT", "bo", "eb", "out")
    )

    persist = ctx.enter_context(tc.tile_pool(name="persist", bufs=1))
    stexp_pool = ctx.enter_context(tc.tile_pool(name="stexp", bufs=4))
    small = ctx.enter_context(tc.tile_pool(name="small", bufs=2))
    outp = ctx.enter_context(tc.tile_pool(name="outp", bufs=4))
    # PSUM budget (8 banks): st_lo 2 + st_hi 2 + ot 2x1 + den 1x1 + drip 1.
    ps_lo = ctx.enter_context(tc.tile_pool(name="ps_lo", bufs=1, space="PSUM"))
    ps_hi = ctx.enter_context(tc.tile_pool(name="ps_hi", bufs=1, space="PSUM"))
    ps_ot = ctx.enter_context(tc.tile_pool(name="ps_ot", bufs=2, space="PSUM"))
    ps_den = ctx.enter_context(tc.tile_pool(name="ps_den", bufs=1, space="PSUM"))
    ps_drip = ctx.enter_context(tc.tile_pool(name="ps_drip", bufs=1, space="PSUM"))

    # ---------- DMAs: everything in flight up front ----------
    # x[b0] + projection weights first (they gate the first score matmuls);
    # the bulky 3.75MB of exp-bias tiles follow (first needed only by the
    # first bias multiply, ~2us after the first exp).
    ones32_sb = persist.tile([128, 32], BF16, tag="ones32", name="ones32")
    nc.vector.memset(ones32_sb[:], 1.0)
    # warm up the exp table set (~2.7us ACT_TABLE_LOAD) under the prologue
    warm = small.tile([1, 32], F32, tag="warm", name="warm_t")
    nc.scalar.activation(out=warm[:], in_=ones32_sb[0:1, :], func=AF.Exp)
    x_sb = [[persist.tile([128, N], BF16, tag=f"x{b}_{cc}", name=f"x{b}_{cc}") for cc in range(2)] for b in range(B_LOC)]
    for cc in range(2):
        nc.sync.dma_start(out=x_sb[0][cc][:], in_=x[0, 128 * cc : 128 * (cc + 1), :])
    wq_sb, wk_sb, wv_sb, wo_sb = [], [], [], []
    for cc in range(2):
        for lst, src, nm in (
            (wq_sb, wqT, "wq"),
            (wk_sb, wkT, "wk"),
            (wv_sb, wvT, "wv"),
            (wo_sb, woT, "wo"),
        ):
            t = persist.tile([128, C], BF16, tag=f"{nm}{cc}", name=f"{nm}{cc}")
            nc.sync.dma_start(out=t[:], in_=src[128 * cc : 128 * (cc + 1), :])
            lst.append(t)
    bo_sb = []
    for cc in range(2):
        t = persist.tile([128, 1], F32, tag=f"bo{cc}", name=f"bo{cc}")
        nc.sync.dma_start(out=t[:], in_=bo[128 * cc : 128 * (cc + 1), :])
        bo_sb.append(t)
    eb_sb = persist.tile([128, HEADS * GW], BF16, tag="eb", name="eb_sb")
    for h in range(HEADS):
        nc.sync.dma_start(out=eb_sb[:, GW * h : GW * (h + 1)], in_=eb[h])
    for cc in range(2):
        nc.sync.dma_start(out=x_sb[1][cc][:], in_=x[1, 128 * cc : 128 * (cc + 1), :])

    # ---------- persistent stage-A outputs ----------
    qT_sb = [[persist.tile([128, N], BF16, tag=f"qT{b}_{oc}", name=f"qT{b}_{oc}") for oc in range(2)] for b in range(B_LOC)]
    kT_sb = [[persist.tile([128, N], BF16, tag=f"kT{b}_{oc}", name=f"kT{b}_{oc}") for oc in range(2)] for b in range(B_LOC)]
    # v: [n, o] layout, 8 row tiles of 128 tokens, ones column per head
    # (33 cols/head) so P@V emits the softmax denominator via ones32 matmuls.
    v_sb = [[persist.tile([128, 33 * HEADS], BF16, tag=f"v{b}_{nt}", name=f"v{b}_{nt}") for nt in range(8)] for b in range(B_LOC)]
    otn_sb = [[persist.tile([128, N], BF16, tag=f"otn{b}_{ch}", name=f"otn{b}_{ch}") for ch in range(2)] for b in range(B_LOC)]

    def emit_qk_group(b, oc, nc2, w_sb, dst, pool_tile):
        for cc in range(2):
            nc.tensor.matmul(
                pool_tile[:, 0:512],
                lhsT=w_sb[cc][:, 128 * oc : 128 * (oc + 1)],
                rhs=x_sb[b][cc][:, 512 * nc2 : 512 * (nc2 + 1)],
                start=(cc == 0),
                stop=(cc == 1),
            )
        nc.vector.tensor_copy(
            out=dst[b][oc][:, 512 * nc2 : 512 * (nc2 + 1)], in_=pool_tile[:, 0:512]
        )

    def emit_v_group(b, nt, pool_tile):
        for cc in range(2):
            nc.tensor.matmul(
                pool_tile[:, 0:C],
                lhsT=x_sb[b][cc][:, 128 * nt : 128 * (nt + 1)],
                rhs=wv_sb[cc][:],
                start=(cc == 0),
                stop=(cc == 1),
            )
        v33 = v_sb[b][nt][:].rearrange("p (h w) -> p h w", w=33)
        nc.vector.tensor_copy(
            out=v33[:, :, 0:32], in_=pool_tile[:, 0:C].rearrange("p (h w) -> p h w", w=32)
        )
        nc.vector.memset(v33[:, :, 32:33], 1.0)

    def stage_c_group(b, ct, q2, pool_tile):
        for ch in range(2):
            nc.tensor.matmul(
                pool_tile[:, 0:512],
                lhsT=wo_sb[ch][:, 128 * ct : 128 * (ct + 1)],
                rhs=otn_sb[b][ch][:, 512 * q2 : 512 * (q2 + 1)],
                start=(ch == 0),
                stop=(ch == 1),
            )
        ob = outp.tile([128, 512], F32, tag="ob", name="ob_t")
        nc.vector.tensor_scalar_add(out=ob[:], in0=pool_tile[:, 0:512], scalar1=bo_sb[ct][:])
        nc.sync.dma_start(
            out=out[b, 128 * ct : 128 * (ct + 1), 512 * q2 : 512 * (q2 + 1)],
            in_=ob[:],
        )

    # ---------- prologue stage A ----------
    # b0's projections run as a dense PE burst at normal priority (they gate
    # the first strips and the burst warms the HAM).  b1's projections and V
    # tiles are emitted at LOW scheduler priority: the Tile scheduler slots
    # them into PE idle slivers during the early strips instead of ahead of
    # the critical score matmuls.
    from contextlib import contextmanager

    @contextmanager
    def lowprio(off):
        tc.cur_priority += off
        try:
            yield
        finally:
            tc.cur_priority -= off

    def drip_tile():
        return ps_drip.tile([128, 512], F32, tag="drip", name="drip_ps")

    pi = 0

    def prol_tile():
        nonlocal pi
        pi += 1
        if pi % 3 == 0:
            return drip_tile()
        return ps_ot.tile([128, 512], F32, tag="ot", name="ot_ps")

    # phase 1 (dense prologue): b0/quad0 q,k at full priority — they gate
    # strip 0 — then b0's V at slightly lower priority (PV consumes v[kt]
    # only from strip 2 onward).
    for nc2 in range(2):
        for w_sb, dst in ((wq_sb, qT_sb), (wk_sb, kT_sb)):
            emit_qk_group(0, 0, nc2, w_sb, dst, prol_tile())
    with lowprio(30):
        for nt in range(8):
            emit_v_group(0, nt, prol_tile())

    # Everything else is dripped INSIDE the strip loop ~10 strips before its
    # deadline (emission position = scheduler priority), one group per drip
    # strip, all through the dedicated drip bank:
    #   qk(b1,oc0) by strip 16; v(b1,nt) by strip 18+nt; qk(b0,oc1) by
    #   strip 32/40; qk(b1,oc1) by strip 48/56; stage C as the otn halves
    #   complete (quad1 norms land ~4 strips into the following qi block).
    drips = {}
    qk_sched = [
        (0, 1, 0, 0), (2, 1, 0, 0, ), (4, 1, 0, 1), (6, 1, 0, 1),
        (23, 0, 1, 0), (25, 0, 1, 0), (27, 0, 1, 1), (29, 0, 1, 1),
        (33, 1, 1, 0), (35, 1, 1, 0), (37, 1, 1, 1), (39, 1, 1, 1),
    ]
    qk_which = {}
    for s, b, oc, nc2 in qk_sched:
        key = (b, oc, nc2)
        w_sb, dst = ((wq_sb, qT_sb), (wk_sb, kT_sb))[qk_which.get(key, 0)]
        qk_which[key] = 1
        drips[s] = (lambda b=b, oc=oc, nc2=nc2, w_sb=w_sb, dst=dst:
                    emit_qk_group(b, oc, nc2, w_sb, dst, drip_tile()))
    for i, s in enumerate((7, 9, 11, 13, 15, 17, 19, 21)):
        drips[s] = (lambda nt=i: emit_v_group(1, nt, drip_tile()))
    for i, (ct,) in enumerate([(0,), (1,)]):
        drips[45 + i] = lambda ct=ct: stage_c_group(0, ct, 0, drip_tile())
        drips[53 + i] = lambda ct=ct: stage_c_group(0, ct, 1, drip_tile())
        drips[60 + i] = lambda ct=ct: stage_c_group(1, ct, 0, drip_tile())

    # ---------- softmax normalization (per qi block) ----------
    eb3 = eb_sb[:].rearrange("p (h w) -> p h w", w=GW)

    def _make_norm(ot_, den_, b_, quad_, qi_):
        # The den accumulator's rows 32h2..32h2+31 hold 32 identical copies
        # of head h2's denominators (M=32 col-tiled ones matmul), i.e. the
        # tile is ALREADY in row-broadcast layout.  The whole normalization
        # is two DVE ops: approx-reciprocal (51 ULP, plenty under the 2e-2
        # budget; denominators are benign positive sums) and one multiply.
        state = {}

        def part1():
            rdb = small.tile([128, 512], F32, tag="rdb", name="rdb_t")
            nc.vector.reciprocal_approx_fast(out=rdb[:], in_=den_[:])
            state["rdb"] = rdb

        def part2():
            nc.vector.tensor_mul(
                out=otn_sb[b_][quad_][:, 512 * qi_ : 512 * (qi_ + 1)],
                in0=ot_[:],
                in1=state["rdb"][:],
            )

        return [part1, part2]

    # ---------- stage B: 64 strips, lag-2 software pipeline ----------
    def emit_pvden(args):
        ot_, den_, b_, quad_, qi_, kt_, se_, first, last = args
        for h2 in range(4):
            nc.tensor.matmul(
                ot_[32 * h2 : 32 * (h2 + 1), :],
                lhsT=v_sb[b_][kt_][:, 33 * (4 * quad_ + h2) : 33 * (4 * quad_ + h2) + 32],
                rhs=se_[:, 512 * h2 : 512 * (h2 + 1)],
                start=first,
                stop=last,
                tile_position=(0, 32 * h2),
                skip_group_check=True,
            )
        for h2 in range(4):
            nc.tensor.matmul(
                den_[32 * h2 : 32 * (h2 + 1), :],
                lhsT=ones32_sb[:],
                rhs=se_[:, 512 * h2 : 512 * (h2 + 1)],
                start=first,
                stop=last,
                tile_position=(0, 32 * h2),
                skip_group_check=True,
            )
        if last:
            norm_parts.extend(_make_norm(ot_, den_, b_, quad_, qi_))

    BLOCKS = [(0, 0), (0, 1), (1, 0), (1, 1)]  # (quad, b)
    pending = deque()
    norm_parts = deque()
    block_acc = {}  # (qi,) accumulators for the current block

    for s in range(64):
        quad, b = BLOCKS[s // 16]
        qi = (s // 8) % 2
        kt = s % 8
        # norm part for a completed qi block (emitted BEFORE this strip's
        # lag-2 PVden so the den bank WAR resolves without a stall)
        if norm_parts:
            norm_parts.popleft()()
        if kt == 0:
            block_acc = (
                ps_ot.tile([128, 512], F32, tag="ot", name="ot_ps"),
                ps_den.tile([128, 512], F32, tag="den", name="den_ps"),
            )
        ot_cur, den_cur = block_acc
        st_lo = ps_lo.tile([128, 1024], F32, tag="stlo", name="stlo_ps")
        st_hi = ps_hi.tile([128, 1024], F32, tag="sthi", name="sthi_ps")
        se = stexp_pool.tile([128, 2048], BF16, tag="se", name="se_t")
        for h2 in range(4):
            dst = st_lo if h2 < 2 else st_hi
            nc.tensor.matmul(
                dst[:, 512 * (h2 % 2) : 512 * (h2 % 2 + 1)],
                lhsT=kT_sb[b][quad][32 * h2 : 32 * (h2 + 1), 128 * kt : 128 * (kt + 1)],
                rhs=qT_sb[b][quad][32 * h2 : 32 * (h2 + 1), 512 * qi : 512 * (qi + 1)],
                start=True,
                stop=True,
                tile_position=(32 * h2, 0),
            )
        nc.scalar.activation(out=se[:, 0:1024], in_=st_lo[:], func=AF.Exp)
        nc.scalar.activation(out=se[:, 1024:2048], in_=st_hi[:], func=AF.Exp)
        off = 896 - 128 * kt + 512 * qi
        nc.vector.tensor_mul(
            out=se[:].rearrange("p (h q) -> p h q", h=4),
            in0=se[:].rearrange("p (h q) -> p h q", h=4),
            in1=eb3[:, 4 * quad : 4 * quad + 4, off : off + 512],
        )
        pending.append((ot_cur, den_cur, b, quad, qi, kt, se, kt == 0, kt == 7))
        if len(pending) > 2:
            emit_pvden(pending.popleft())
        if s in drips:
            drips[s]()
        else:
            # HAM keep-warm filler: ~500ns of junk matmul into the idle drip
            # bank.  Real per-strip PE work is ~1560ns warm vs the 2294ns exp
            # cadence (68% duty) — low enough that the HAM re-throttles the
            # PE to 1.2 GHz, where the same work is 2477ns and over budget.
            # The filler keeps duty >90% so the array stays at 2.4 GHz; on
            # drip strips the dripped projection group plays this role.
            with lowprio(60):
                fill = drip_tile()
                for j in range(2):
                    nc.tensor.matmul(
                        fill[0:32, :],
                        lhsT=ones32_sb[:],
                        rhs=x_sb[0][0][:, 512 * j : 512 * (j + 1)],
                        start=True,
                        stop=True,
                    )

    # ---------- tail ----------
    while pending:
        emit_pvden(pending.popleft())
    while norm_parts:
        norm_parts.popleft()()
    # the two final output-projection groups on separate banks so they run
    # concurrently
    stage_c_group(1, 0, 1, drip_tile())
    stage_c_group(1, 1, 1, ps_ot.tile([128, 512], F32, tag="ot", name="ot_ps"))


def build():
    nc = bacc.Bacc("TRN2", target_bir_lowering=False, debug=False, num_devices=N_CORES)
    io = {
        "x": nc.dram_tensor("x", [B_LOC, C, N], BF16, kind="ExternalInput").ap(),
        "wqT": nc.dram_tensor("wqT", [C, C], BF16, kind="ExternalInput").ap(),
        "wkT": nc.dram_tensor("wkT", [C, C], BF16, kind="ExternalInput").ap(),
        "wvT": nc.dram_tensor("wvT", [C, C], BF16, kind="ExternalInput").ap(),
        "woT": nc.dram_tensor("woT", [C, C], BF16, kind="ExternalInput").ap(),
        "bo": nc.dram_tensor("bo", [C, 1], F32, kind="ExternalInput").ap(),
        "eb": nc.dram_tensor("eb", [HEADS, 128, GW], BF16, kind="ExternalInput").ap(),
        "out": nc.dram_tensor("out", [B_LOC, C, N], F32, kind="ExternalOutput").ap(),
    }
    with tile.TileContext(nc) as tc:
        _emit(tc, io)
    nc.compile()
    return nc


_CACHE: dict = {}


def _get_nc():
    if "nc" not in _CACHE:
        _CACHE["nc"] = build()
    return _CACHE["nc"]


def make_in_maps(x, Wq, Wk, Wv, Wo, bo, rel_bias, rel_idx=None):
    """Host-side sharding/layout prep. Returns per-core input maps."""
    import ml_dtypes

    bf16 = ml_dtypes.bfloat16
    x = np.asarray(x, np.float32)
    b, c, H, W = x.shape
    assert (b, c, H * W) == (B_LOC * N_CORES, C, N)
    xr = np.ascontiguousarray(x.reshape(b, c, N).astype(bf16))
    wqT = np.ascontiguousarray(np.asarray(Wq, np.float32).T.astype(bf16))
    wkT = np.ascontiguousarray((np.asarray(Wk, np.float32) * SCALE).T.astype(bf16))
    wvT = np.ascontiguousarray(np.asarray(Wv, np.float32).T.astype(bf16))
    woT = np.ascontiguousarray(np.asarray(Wo, np.float32).T.astype(bf16))
    bo2 = np.ascontiguousarray(np.asarray(bo, np.float32).reshape(C, 1))
    rb = np.asarray(rel_bias, np.float32)
    idx = G0 + np.arange(128)[:, None] - np.arange(GW)[None, :]
    ebmat = np.ascontiguousarray(np.exp(rb[:, idx]).astype(bf16))  # [8, 128, GW]
    shared = dict(wqT=wqT, wkT=wkT, wvT=wvT, woT=woT, bo=bo2, eb=ebmat)
    return [
        dict(x=np.ascontiguousarray(xr[B_LOC * i : B_LOC * (i + 1)]), **shared)
        for i in range(N_CORES)
    ]


def _install_ntff_hook_shim():
    """bass_utils fetches the axon NTFF hook via antenv.axon_hooks, which this
    image's antenv lacks; synthesize it from trn_agent_boot's ctypes hook."""
    import sys
    import types

    try:
        from antenv.axon_hooks import get_axon_ntff_profile_hook  # noqa: F401

        return
    except ImportError:
        pass
    hook = None
    try:
        from trn_agent_boot.trn_boot import _ntff_profile_via_ctypes

        hook = _ntff_profile_via_ctypes("/opt/axon/libaxon_pjrt.so")
    except Exception:
        pass
    mod = types.ModuleType("antenv.axon_hooks")
    state = {"hook": hook}
    mod.get_axon_ntff_profile_hook = lambda: state["hook"]
    mod.set_axon_ntff_profile_hook = lambda h: state.__setitem__("hook", h)
    sys.modules["antenv.axon_hooks"] = mod


def run(inputs: dict, trace: bool = False):
    """Run on the 8 cores; returns (full_output, BassKernelResults)."""
    if trace:
        _install_ntff_hook_shim()
    in_maps = make_in_maps(**inputs)
    nc = _get_nc()
    res = bass_utils.run_bass_kernel_spmd(
        nc, in_maps, core_ids=list(range(N_CORES)), trace=trace
    )
    outs = np.stack([res.results[i]["out"] for i in range(N_CORES)])
    out = outs.reshape(B_LOC * N_CORES, C, 32, 32)
    return out, res


def kernel(**inputs) -> np.ndarray:
    out, _ = run(inputs)
    return out


# revision 20
# speedup vs baseline: 1.1250x; 1.1250x over previous
"""CoAtNet relative attention kernel for Trainium2 (Bass/Tile), 8 NeuronCores.

Problem (per full input):
  x [16, 256, 32, 32] f32; Wq/Wk/Wv [256, 256]; Wo [256, 256]; bo [256];
  rel_bias [8, 3969]; rel_idx [1024, 1024] int32 (static pattern).
  out[b] = softmax(q k^T / sqrt(d) + bias) v  projected back, heads=8, d=32.

Sharding: data-parallel over batch — each of the 8 cores handles 2 batches
with identical programs (SPMD, no collectives).

Key structural facts used:
  * rel_idx[p, q] == (q - p) + 1056 exactly (the reference's quirky *W stride
    collapses the 2D relative index to 1D Toeplitz).  So the [1024, 1024]
    bias matrix per head is bias[p, q] = rel_bias[h, q - p + 1056] and any
    [128, width] tile of it (keys on partitions) is a contiguous slice of a
    small "sheared" tile  G[h, i, j'] = rel_bias[h, 1952 + i - j']  of shape
    [128, 1920].  No gather on device at all.  The bias is applied as
    exp(S+B) = exp(S) * exp(B) with exp(B) precomputed, so the application
    is a bf16 2x-mode multiply instead of an fp32 1x add.
  * Everything is computed in "transposed" layout so no transposes are ever
    needed: x arrives as [c, n] per batch; Q^T/K^T = W @ x are [d_all, n];
    scores are built as S^T [keys, queries]; P@V uses lhsT = V directly;
    and the final projection produces out^T [c, n], exactly the output
    memory layout.
  * The kernel is ACT(exp)-throughput-bound: 16.8M exps/core at 1 elem/
    lane/cycle @1.2GHz is a ~110us floor.  The schedule is built so the
    ScalarE never waits: each strip's exp is SPLIT into two ACTIVATEs over
    separate PSUM tiles (st_lo banks 0-1, st_hi banks 2-3) so the next
    strip's score matmuls can overwrite the low banks while ACT still
    processes the high banks (the single-buffered [128,2048] fp32 score
    tile cannot be double-buffered: TRN2 matmuls write fp32-only and PSUM
    has just 8 banks).  PV/den matmuls run TWO strips behind the score
    matmuls so a DVE-gated PV never head-of-line blocks ready ST work in
    the PE FIFO.  All projection / output-projection / softmax-
    normalization work is dripped into per-strip slack slots on the other
    engines.
"""

import numpy as np
from collections import deque
from contextlib import ExitStack

import concourse.bass as bass
import concourse.bacc as bacc
import concourse.mybir as mybir
import concourse.tile as tile
from concourse import bass_utils
from concourse._compat import with_exitstack

HEADS = 8
D = 32  # head dim
C = 256  # channels = heads * D
N = 1024  # tokens = 32 * 32
B_LOC = 2  # batches per core
N_CORES = 8
SCALE = D ** -0.5
GW = 1920  # sheared bias tile width
G0 = 1952  # G[h, i, j'] = rel_bias[h, G0 + i - j']

F32 = mybir.dt.float32
BF16 = mybir.dt.bfloat16
AF = mybir.ActivationFunctionType


@with_exitstack
def _emit(ctx: ExitStack, tc: tile.TileContext, io: dict):
    nc = tc.nc
    x, wqT, wkT, wvT, woT, bo, eb, out = (
        io[k] for k in ("x", "wqT", "wkT", "wvT", "woT", "bo", "eb", "out")
    )

    persist = ctx.enter_context(tc.tile_pool(name="persist", bufs=1))
    stexp_pool = ctx.enter_context(tc.tile_pool(name="stexp", bufs=4))
    small = ctx.enter_context(tc.tile_pool(name="small", bufs=2))
    outp = ctx.enter_context(tc.tile_pool(name="outp", bufs=4))
    # PSUM budget (8 banks): st_lo 2 + st_hi 2 + ot 2x1 + den 1x1 + drip 1.
    ps_lo = ctx.enter_context(tc.tile_pool(name="ps_lo", bufs=1, space="PSUM"))
    ps_hi = ctx.enter_context(tc.tile_pool(name="ps_hi", bufs=1, space="PSUM"))
    ps_ot = ctx.enter_context(tc.tile_pool(name="ps_ot", bufs=2, space="PSUM"))
    ps_den = ctx.enter_context(tc.tile_pool(name="ps_den", bufs=1, space="PSUM"))
    ps_drip = ctx.enter_context(tc.tile_pool(name="ps_drip", bufs=1, space="PSUM"))

    # ---------- DMAs: everything in flight up front ----------
    # x[b0] + projection weights first (they gate the first score matmuls);
    # the bulky 3.75MB of exp-bias tiles follow (first needed only by the
    # first bias multiply, ~2us after the first exp).
    ones32_sb = persist.tile([128, 32], BF16, tag="ones32", name="ones32")
    nc.vector.memset(ones32_sb[:], 1.0)
    # warm up the exp table set (~2.7us ACT_TABLE_LOAD) under the prologue
    warm = small.tile([1, 32], F32, tag="warm", name="warm_t")
    nc.scalar.activation(out=warm[:], in_=ones32_sb[0:1, :], func=AF.Exp)
    x_sb = [[persist.tile([128, N], BF16, tag=f"x{b}_{cc}", name=f"x{b}_{cc}") for cc in range(2)] for b in range(B_LOC)]
    for cc in range(2):
        nc.sync.dma_start(out=x_sb[0][cc][:], in_=x[0, 128 * cc : 128 * (cc + 1), :])
    wq_sb, wk_sb, wv_sb, wo_sb = [], [], [], []
    for cc in range(2):
        for lst, src, nm in (
            (wq_sb, wqT, "wq"),
            (wk_sb, wkT, "wk"),
            (wv_sb, wvT, "wv"),
            (wo_sb, woT, "wo"),
        ):
            t = persist.tile([128, C], BF16, tag=f"{nm}{cc}", name=f"{nm}{cc}")
            nc.sync.dma_start(out=t[:], in_=src[128 * cc : 128 * (cc + 1), :])
            lst.append(t)
    bo_sb = []
    for cc in range(2):
        t = persist.tile([128, 1], F32, tag=f"bo{cc}", name=f"bo{cc}")
        nc.sync.dma_start(out=t[:], in_=bo[128 * cc : 128 * (cc + 1), :])
        bo_sb.append(t)
    eb_sb = persist.tile([128, HEADS * GW], BF16, tag="eb", name="eb_sb")
    for h in range(HEADS):
        nc.sync.dma_start(out=eb_sb[:, GW * h : GW * (h + 1)], in_=eb[h])
    for cc in range(2):
        nc.sync.dma_start(out=x_sb[1][cc][:], in_=x[1, 128 * cc : 128 * (cc + 1), :])

    # ---------- persistent stage-A outputs ----------
    qT_sb = [[persist.tile([128, N], BF16, tag=f"qT{b}_{oc}", name=f"qT{b}_{oc}") for oc in range(2)] for b in range(B_LOC)]
    kT_sb = [[persist.tile([128, N], BF16, tag=f"kT{b}_{oc}", name=f"kT{b}_{oc}") for oc in range(2)] for b in range(B_LOC)]
    # v: [n, o] layout, 8 row tiles of 128 tokens, ones column per head
    # (33 cols/head) so P@V emits the softmax denominator via ones32 matmuls.
    v_sb = [[persist.tile([128, 33 * HEADS], BF16, tag=f"v{b}_{nt}", name=f"v{b}_{nt}") for nt in range(8)] for b in range(B_LOC)]
    otn_sb = [[persist.tile([128, N], BF16, tag=f"otn{b}_{ch}", name=f"otn{b}_{ch}") for ch in range(2)] for b in range(B_LOC)]

    def emit_qk_group(b, oc, nc2, w_sb, dst, pool_tile):
        for cc in range(2):
            nc.tensor.matmul(
                pool_tile[:, 0:512],
                lhsT=w_sb[cc][:, 128 * oc : 128 * (oc + 1)],
                rhs=x_sb[b][cc][:, 512 * nc2 : 512 * (nc2 + 1)],
                start=(cc == 0),
                stop=(cc == 1),
            )
        nc.vector.tensor_copy(
            out=dst[b][oc][:, 512 * nc2 : 512 * (nc2 + 1)], in_=pool_tile[:, 0:512]
        )

    def emit_v_group(b, nt, pool_tile):
        for cc in range(2):
            nc.tensor.matmul(
                pool_tile[:, 0:C],
                lhsT=x_sb[b][cc][:, 128 * nt : 128 * (nt + 1)],
                rhs=wv_sb[cc][:],
                start=(cc == 0),
                stop=(cc == 1),
            )
        v33 = v_sb[b][nt][:].rearrange("p (h w) -> p h w", w=33)
        nc.vector.tensor_copy(
            out=v33[:, :, 0:32], in_=pool_tile[:, 0:C].rearrange("p (h w) -> p h w", w=32)
        )
        nc.vector.memset(v33[:, :, 32:33], 1.0)

    def stage_c_group(b, ct, q2, pool_tile):
        for ch in range(2):
            nc.tensor.matmul(
                pool_tile[:, 0:512],
                lhsT=wo_sb[ch][:, 128 * ct : 128 * (ct + 1)],
                rhs=otn_sb[b][ch][:, 512 * q2 : 512 * (q2 + 1)],
                start=(ch == 0),
                stop=(ch == 1),
            )
        ob = outp.tile([128, 512], BF16, tag="ob", name="ob_t")
        nc.vector.tensor_scalar_add(out=ob[:], in0=pool_tile[:, 0:512], scalar1=bo_sb[ct][:])
        nc.sync.dma_start(
            out=out[b, 128 * ct : 128 * (ct + 1), 512 * q2 : 512 * (q2 + 1)],
            in_=ob[:],
        )

    # ---------- prologue stage A ----------
    # b0's projections run as a dense PE burst at normal priority (they gate
    # the first strips and the burst warms the HAM).  b1's projections and V
    # tiles are emitted at LOW scheduler priority: the Tile scheduler slots
    # them into PE idle slivers during the early strips instead of ahead of
    # the critical score matmuls.
    from contextlib import contextmanager

    @contextmanager
    def lowprio(off):
        tc.cur_priority += off
        try:
            yield
        finally:
            tc.cur_priority -= off

    def drip_tile():
        return ps_drip.tile([128, 512], F32, tag="drip", name="drip_ps")

    pi = 0

    def prol_tile():
        nonlocal pi
        pi += 1
        if pi % 3 == 0:
            return drip_tile()
        return ps_ot.tile([128, 512], F32, tag="ot", name="ot_ps")

    # phase 1 (dense prologue): b0/quad0 q,k at full priority — they gate
    # strip 0 — then b0's V at slightly lower priority (PV consumes v[kt]
    # only from strip 2 onward).
    for nc2 in range(2):
        for w_sb, dst in ((wq_sb, qT_sb), (wk_sb, kT_sb)):
            emit_qk_group(0, 0, nc2, w_sb, dst, prol_tile())
    with lowprio(30):
        emit_v_group(0, 0, prol_tile())

    # Everything else is dripped INSIDE the strip loop ~10 strips before its
    # deadline (emission position = scheduler priority), one group per drip
    # strip, all through the dedicated drip bank:
    #   qk(b1,oc0) by strip 16; v(b1,nt) by strip 18+nt; qk(b0,oc1) by
    #   strip 32/40; qk(b1,oc1) by strip 48/56; stage C as the otn halves
    #   complete (quad1 norms land ~4 strips into the following qi block).
    drips = {}
    # v(b0, 1..7): v[nt] consumed by the PVden emitted at strip nt+2
    for i, s in enumerate((0, 1, 2, 3, 4, 5, 6)):
        drips[s] = (lambda nt=i + 1: emit_v_group(0, nt, drip_tile()))
    qk_sched = [
        (8, 1, 0, 0), (10, 1, 0, 0), (12, 1, 0, 1), (14, 1, 0, 1),
        (25, 0, 1, 0), (27, 0, 1, 0), (29, 0, 1, 1), (31, 0, 1, 1),
        (33, 1, 1, 0), (35, 1, 1, 0), (37, 1, 1, 1), (39, 1, 1, 1),
    ]
    qk_which = {}
    for s, b, oc, nc2 in qk_sched:
        key = (b, oc, nc2)
        w_sb, dst = ((wq_sb, qT_sb), (wk_sb, kT_sb))[qk_which.get(key, 0)]
        qk_which[key] = 1
        drips[s] = (lambda b=b, oc=oc, nc2=nc2, w_sb=w_sb, dst=dst:
                    emit_qk_group(b, oc, nc2, w_sb, dst, drip_tile()))
    for i, s in enumerate((16, 17, 18, 19, 20, 21, 22, 23)):
        drips[s] = (lambda nt=i: emit_v_group(1, nt, drip_tile()))
    for i, (ct,) in enumerate([(0,), (1,)]):
        drips[45 + i] = lambda ct=ct: stage_c_group(0, ct, 0, drip_tile())
        drips[53 + i] = lambda ct=ct: stage_c_group(0, ct, 1, drip_tile())
        drips[60 + i] = lambda ct=ct: stage_c_group(1, ct, 0, drip_tile())

    # ---------- softmax normalization (per qi block) ----------
    eb3 = eb_sb[:].rearrange("p (h w) -> p h w", w=GW)

    def _make_norm(ot_, den_, b_, quad_, qi_):
        # The den accumulator's rows 32h2..32h2+31 hold 32 identical copies
        # of head h2's denominators (M=32 col-tiled ones matmul), i.e. the
        # tile is ALREADY in row-broadcast layout.  The whole normalization
        # is two DVE ops: approx-reciprocal (51 ULP, plenty under the 2e-2
        # budget; denominators are benign positive sums) and one multiply.
        state = {}

        def part1():
            rdb = small.tile([128, 512], F32, tag="rdb", name="rdb_t")
            nc.vector.reciprocal_approx_fast(out=rdb[:], in_=den_[:])
            state["rdb"] = rdb

        def part2():
            nc.vector.tensor_mul(
                out=otn_sb[b_][quad_][:, 512 * qi_ : 512 * (qi_ + 1)],
                in0=ot_[:],
                in1=state["rdb"][:],
            )

        return [part1, part2]

    # ---------- stage B: 64 strips, lag-2 software pipeline ----------
    def emit_pvden(args):
        ot_, den_, b_, quad_, qi_, kt_, se_, first, last = args
        for h2 in range(4):
            nc.tensor.matmul(
                ot_[32 * h2 : 32 * (h2 + 1), :],
                lhsT=v_sb[b_][kt_][:, 33 * (4 * quad_ + h2) : 33 * (4 * quad_ + h2) + 32],
                rhs=se_[:, 512 * h2 : 512 * (h2 + 1)],
                start=first,
                stop=last,
                tile_position=(0, 32 * h2),
                skip_group_check=True,
            )
        for h2 in range(4):
            nc.tensor.matmul(
                den_[32 * h2 : 32 * (h2 + 1), :],
                lhsT=ones32_sb[:],
                rhs=se_[:, 512 * h2 : 512 * (h2 + 1)],
                start=first,
                stop=last,
                tile_position=(0, 32 * h2),
                skip_group_check=True,
            )
        if last:
            norm_parts.extend(_make_norm(ot_, den_, b_, quad_, qi_))

    BLOCKS = [(0, 0), (0, 1), (1, 0), (1, 1)]  # (quad, b)
    pending = deque()
    norm_parts = deque()
    block_acc = {}  # (qi,) accumulators for the current block

    for s in range(64):
        quad, b = BLOCKS[s // 16]
        qi = (s // 8) % 2
        kt = s % 8
        # norm part for a completed qi block (emitted BEFORE this strip's
        # lag-2 PVden so the den bank WAR resolves without a stall)
        if norm_parts:
            norm_parts.popleft()()
        if kt == 0:
            block_acc = (
                ps_ot.tile([128, 512], F32, tag="ot", name="ot_ps"),
                ps_den.tile([128, 512], F32, tag="den", name="den_ps"),
            )
        ot_cur, den_cur = block_acc
        st_lo = ps_lo.tile([128, 1024], F32, tag="stlo", name="stlo_ps")
        st_hi = ps_hi.tile([128, 1024], F32, tag="sthi", name="sthi_ps")
        se = stexp_pool.tile([128, 2048], BF16, tag="se", name="se_t")
        for h2 in range(4):
            dst = st_lo if h2 < 2 else st_hi
            nc.tensor.matmul(
                dst[:, 512 * (h2 % 2) : 512 * (h2 % 2 + 1)],
                lhsT=kT_sb[b][quad][32 * h2 : 32 * (h2 + 1), 128 * kt : 128 * (kt + 1)],
                rhs=qT_sb[b][quad][32 * h2 : 32 * (h2 + 1), 512 * qi : 512 * (qi + 1)],
                start=True,
                stop=True,
                tile_position=(32 * h2, 0),
            )
        nc.scalar.activation(out=se[:, 0:1024], in_=st_lo[:], func=AF.Exp)
        nc.scalar.activation(out=se[:, 1024:2048], in_=st_hi[:], func=AF.Exp)
        off = 896 - 128 * kt + 512 * qi
        nc.vector.tensor_mul(
            out=se[:].rearrange("p (h q) -> p h q", h=4),
            in0=se[:].rearrange("p (h q) -> p h q", h=4),
            in1=eb3[:, 4 * quad : 4 * quad + 4, off : off + 512],
        )
        pending.append((ot_cur, den_cur, b, quad, qi, kt, se, kt == 0, kt == 7))
        if len(pending) > 2:
            emit_pvden(pending.popleft())
        if s in drips:
            drips[s]()

    # ---------- tail ----------
    while pending:
        emit_pvden(pending.popleft())
    while norm_parts:
        norm_parts.popleft()()
    # the two final output-projection groups on separate banks so they run
    # concurrently
    stage_c_group(1, 0, 1, drip_tile())
    stage_c_group(1, 1, 1, ps_ot.tile([128, 512], F32, tag="ot", name="ot_ps"))


def build():
    nc = bacc.Bacc("TRN2", target_bir_lowering=False, debug=False, num_devices=N_CORES)
    io = {
        "x": nc.dram_tensor("x", [B_LOC, C, N], BF16, kind="ExternalInput").ap(),
        "wqT": nc.dram_tensor("wqT", [C, C], BF16, kind="ExternalInput").ap(),
        "wkT": nc.dram_tensor("wkT", [C, C], BF16, kind="ExternalInput").ap(),
        "wvT": nc.dram_tensor("wvT", [C, C], BF16, kind="ExternalInput").ap(),
        "woT": nc.dram_tensor("woT", [C, C], BF16, kind="ExternalInput").ap(),
        "bo": nc.dram_tensor("bo", [C, 1], F32, kind="ExternalInput").ap(),
        "eb": nc.dram_tensor("eb", [HEADS, 128, GW], BF16, kind="ExternalInput").ap(),
        "out": nc.dram_tensor("out", [B_LOC, C, N], BF16, kind="ExternalOutput").ap(),
    }
    with tile.TileContext(nc) as tc:
        _emit(tc, io)
    nc.compile()
    return nc


_CACHE: dict = {}


def _get_nc():
    if "nc" not in _CACHE:
        _CACHE["nc"] = build()
    return _CACHE["nc"]


def make_in_maps(x, Wq, Wk, Wv, Wo, bo, rel_bias, rel_idx=None):
    """Host-side sharding/layout prep. Returns per-core input maps."""
    import ml_dtypes

    bf16 = ml_dtypes.bfloat16
    x = np.asarray(x, np.float32)
    b, c, H, W = x.shape
    assert (b, c, H * W) == (B_LOC * N_CORES, C, N)
    xr = np.ascontiguousarray(x.reshape(b, c, N).astype(bf16))
    wqT = np.ascontiguousarray(np.asarray(Wq, np.float32).T.astype(bf16))
    wkT = np.ascontiguousarray((np.asarray(Wk, np.float32) * SCALE).T.astype(bf16))
    wvT = np.ascontiguousarray(np.asarray(Wv, np.float32).T.astype(bf16))
    woT = np.ascontiguousarray(np.asarray(Wo, np.float32).T.astype(bf16))
    bo2 = np.ascontiguousarray(np.asarray(bo, np.float32).reshape(C, 1))
    rb = np.asarray(rel_bias, np.float32)
    idx = G0 + np.arange(128)[:, None] - np.arange(GW)[None, :]
    ebmat = np.ascontiguousarray(np.exp(rb[:, idx]).astype(bf16))  # [8, 128, GW]
    shared = dict(wqT=wqT, wkT=wkT, wvT=wvT, woT=woT, bo=bo2, eb=ebmat)
    return [
        dict(x=np.ascontiguousarray(xr[B_LOC * i : B_LOC * (i + 1)]), **shared)
        for i in range(N_CORES)
    ]


def _install_ntff_hook_shim():
    """bass_utils fetches the axon NTFF hook via antenv.axon_hooks, which this
    image's antenv lacks; synthesize it from trn_agent_boot's ctypes hook."""
    import sys
    import types

    try:
        from antenv.axon_hooks import get_axon_ntff_profile_hook  # noqa: F401

        return
    except ImportError:
        pass
    hook = None
    try:
        from trn_agent_boot.trn_boot import _ntff_profile_via_ctypes

        hook = _ntff_profile_via_ctypes("/opt/axon/libaxon_pjrt.so")
    except Exception:
        pass
    mod = types.ModuleType("antenv.axon_hooks")
    state = {"hook": hook}
    mod.get_axon_ntff_profile_hook = lambda: state["hook"]
    mod.set_axon_ntff_profile_hook = lambda h: state.__setitem__("hook", h)
    sys.modules["antenv.axon_hooks"] = mod


def run(inputs: dict, trace: bool = False):
    """Run on the 8 cores; returns (full_output, BassKernelResults)."""
    if trace:
        _install_ntff_hook_shim()
    in_maps = make_in_maps(**inputs)
    nc = _get_nc()
    res = bass_utils.run_bass_kernel_spmd(
        nc, in_maps, core_ids=list(range(N_CORES)), trace=trace
    )
    outs = np.stack([np.asarray(res.results[i]["out"], np.float32) for i in range(N_CORES)])
    out = outs.reshape(B_LOC * N_CORES, C, 32, 32)
    return out, res


def kernel(**inputs) -> np.ndarray:
    out, _ = run(inputs)
    return out


# revision 21
# speedup vs baseline: 1.1290x; 1.0036x over previous
"""CoAtNet relative attention kernel for Trainium2 (Bass/Tile), 8 NeuronCores.

Problem (per full input):
  x [16, 256, 32, 32] f32; Wq/Wk/Wv [256, 256]; Wo [256, 256]; bo [256];
  rel_bias [8, 3969]; rel_idx [1024, 1024] int32 (static pattern).
  out[b] = softmax(q k^T / sqrt(d) + bias) v  projected back, heads=8, d=32.

Sharding: data-parallel over batch — each of the 8 cores handles 2 batches
with identical programs (SPMD, no collectives).

Key structural facts used:
  * rel_idx[p, q] == (q - p) + 1056 exactly (the reference's quirky *W stride
    collapses the 2D relative index to 1D Toeplitz).  So the [1024, 1024]
    bias matrix per head is bias[p, q] = rel_bias[h, q - p + 1056] and any
    [128, width] tile of it (keys on partitions) is a contiguous slice of a
    small "sheared" tile  G[h, i, j'] = rel_bias[h, 1952 + i - j']  of shape
    [128, 1920].  No gather on device at all.  The bias is applied as
    exp(S+B) = exp(S) * exp(B) with exp(B) precomputed, so the application
    is a bf16 2x-mode multiply instead of an fp32 1x add.
  * Everything is computed in "transposed" layout so no transposes are ever
    needed: x arrives as [c, n] per batch; Q^T/K^T = W @ x are [d_all, n];
    scores are built as S^T [keys, queries]; P@V uses lhsT = V directly;
    and the final projection produces out^T [c, n], exactly the output
    memory layout.
  * The kernel is ACT(exp)-throughput-bound: 16.8M exps/core at 1 elem/
    lane/cycle @1.2GHz is a ~110us floor.  The schedule is built so the
    ScalarE never waits: each strip's exp is SPLIT into two ACTIVATEs over
    separate PSUM tiles (st_lo banks 0-1, st_hi banks 2-3) so the next
    strip's score matmuls can overwrite the low banks while ACT still
    processes the high banks (the single-buffered [128,2048] fp32 score
    tile cannot be double-buffered: TRN2 matmuls write fp32-only and PSUM
    has just 8 banks).  PV/den matmuls run TWO strips behind the score
    matmuls so a DVE-gated PV never head-of-line blocks ready ST work in
    the PE FIFO.  All projection / output-projection / softmax-
    normalization work is dripped into per-strip slack slots on the other
    engines.
"""

import numpy as np
from collections import deque
from contextlib import ExitStack

import concourse.bass as bass
import concourse.bacc as bacc
import concourse.mybir as mybir
import concourse.tile as tile
from concourse import bass_utils
from concourse._compat import with_exitstack

HEADS = 8
D = 32  # head dim
C = 256  # channels = heads * D
N = 1024  # tokens = 32 * 32
B_LOC = 2  # batches per core
N_CORES = 8
SCALE = D ** -0.5
GW = 1920  # sheared bias tile width
G0 = 1952  # G[h, i, j'] = rel_bias[h, G0 + i - j']

F32 = mybir.dt.float32
BF16 = mybir.dt.bfloat16
AF = mybir.ActivationFunctionType


@with_exitstack
def _emit(ctx: ExitStack, tc: tile.TileContext, io: dict):
    nc = tc.nc
    x, wqT, wkT, wvT, woT, bo, eb, out = (
        io[k] for k in ("x", "wqT", "wkT", "wvT", "woT", "bo", "eb", "out")
    )

    persist = ctx.enter_context(tc.tile_pool(name="persist", bufs=1))
    stexp_pool = ctx.enter_context(tc.tile_pool(name="stexp", bufs=4))
    small = ctx.enter_context(tc.tile_pool(name="small", bufs=2))
    outp = ctx.enter_context(tc.tile_pool(name="outp", bufs=4))
    # PSUM budget (8 banks): st_lo 2 + st_hi 2 + ot 2x1 + den 1x1 + drip 1.
    ps_lo = ctx.enter_context(tc.tile_pool(name="ps_lo", bufs=1, space="PSUM"))
    ps_hi = ctx.enter_context(tc.tile_pool(name="ps_hi", bufs=1, space="PSUM"))
    ps_ot = ctx.enter_context(tc.tile_pool(name="ps_ot", bufs=2, space="PSUM"))
    ps_den = ctx.enter_context(tc.tile_pool(name="ps_den", bufs=1, space="PSUM"))
    ps_drip = ctx.enter_context(tc.tile_pool(name="ps_drip", bufs=1, space="PSUM"))

    # ---------- DMAs: everything in flight up front ----------
    # x[b0] + projection weights first (they gate the first score matmuls);
    # the bulky 3.75MB of exp-bias tiles follow (first needed only by the
    # first bias multiply, ~2us after the first exp).
    ones32_sb = persist.tile([128, 32], BF16, tag="ones32", name="ones32")
    nc.vector.memset(ones32_sb[:], 1.0)
    # warm up the exp table set (~2.7us ACT_TABLE_LOAD) under the prologue
    warm = small.tile([1, 32], F32, tag="warm", name="warm_t")
    nc.scalar.activation(out=warm[:], in_=ones32_sb[0:1, :], func=AF.Exp)
    x_sb = [persist.tile([128, 2 * N], BF16, tag=f"x{b}", name=f"x{b}") for b in range(B_LOC)]
    nc.sync.dma_start(
        out=x_sb[0][:].rearrange("p (cc q) -> p cc q", q=N),
        in_=x[0].rearrange("(cc p) q -> p cc q", p=128),
    )
    w_tiles = {}
    for wsrc, nm in ((wqT, "wq"), (wkT, "wk"), (wvT, "wv"), (woT, "wo")):
        t = persist.tile([128, 2 * C], BF16, tag=nm, name=nm)
        nc.sync.dma_start(
            out=t[:].rearrange("p (cc q) -> p cc q", q=C),
            in_=wsrc.rearrange("(cc p) q -> p cc q", p=128),
        )
        w_tiles[nm] = t
    wq_sb, wk_sb, wv_sb, wo_sb = (w_tiles[n] for n in ("wq", "wk", "wv", "wo"))
    bo_sb = persist.tile([128, 2], F32, tag="bo", name="bo")
    nc.sync.dma_start(
        out=bo_sb[:], in_=bo.rearrange("(cc p) one -> p (cc one)", p=128)
    )
    eb_sb = persist.tile([128, HEADS * GW], BF16, tag="eb", name="eb_sb")
    for h in range(HEADS):
        nc.sync.dma_start(out=eb_sb[:, GW * h : GW * (h + 1)], in_=eb[h])
    nc.sync.dma_start(
        out=x_sb[1][:].rearrange("p (cc q) -> p cc q", q=N),
        in_=x[1].rearrange("(cc p) q -> p cc q", p=128),
    )

    # ---------- persistent stage-A outputs ----------
    qT_sb = [[persist.tile([128, N], BF16, tag=f"qT{b}_{oc}", name=f"qT{b}_{oc}") for oc in range(2)] for b in range(B_LOC)]
    kT_sb = [[persist.tile([128, N], BF16, tag=f"kT{b}_{oc}", name=f"kT{b}_{oc}") for oc in range(2)] for b in range(B_LOC)]
    # v: [n, o] layout, 8 row tiles of 128 tokens, ones column per head
    # (33 cols/head) so P@V emits the softmax denominator via ones32 matmuls.
    v_sb = [[persist.tile([128, 33 * HEADS], BF16, tag=f"v{b}_{nt}", name=f"v{b}_{nt}") for nt in range(8)] for b in range(B_LOC)]
    otn_sb = [[persist.tile([128, N], BF16, tag=f"otn{b}_{ch}", name=f"otn{b}_{ch}") for ch in range(2)] for b in range(B_LOC)]

    def emit_qk_group(b, oc, nc2, w_sb, dst, pool_tile):
        for cc in range(2):
            nc.tensor.matmul(
                pool_tile[:, 0:512],
                lhsT=w_sb[:, 256 * cc + 128 * oc : 256 * cc + 128 * (oc + 1)],
                rhs=x_sb[b][:, 1024 * cc + 512 * nc2 : 1024 * cc + 512 * (nc2 + 1)],
                start=(cc == 0),
                stop=(cc == 1),
            )
        nc.vector.tensor_copy(
            out=dst[b][oc][:, 512 * nc2 : 512 * (nc2 + 1)], in_=pool_tile[:, 0:512]
        )

    def emit_v_group(b, nt, pool_tile):
        for cc in range(2):
            nc.tensor.matmul(
                pool_tile[:, 0:C],
                lhsT=x_sb[b][:, 1024 * cc + 128 * nt : 1024 * cc + 128 * (nt + 1)],
                rhs=wv_sb[:, 256 * cc : 256 * (cc + 1)],
                start=(cc == 0),
                stop=(cc == 1),
            )
        v33 = v_sb[b][nt][:].rearrange("p (h w) -> p h w", w=33)
        nc.vector.tensor_copy(
            out=v33[:, :, 0:32], in_=pool_tile[:, 0:C].rearrange("p (h w) -> p h w", w=32)
        )
        nc.vector.memset(v33[:, :, 32:33], 1.0)

    def stage_c_group(b, ct, q2, pool_tile):
        for ch in range(2):
            nc.tensor.matmul(
                pool_tile[:, 0:512],
                lhsT=wo_sb[:, 256 * ch + 128 * ct : 256 * ch + 128 * (ct + 1)],
                rhs=otn_sb[b][ch][:, 512 * q2 : 512 * (q2 + 1)],
                start=(ch == 0),
                stop=(ch == 1),
            )
        ob = outp.tile([128, 512], BF16, tag="ob", name="ob_t")
        nc.vector.tensor_scalar_add(out=ob[:], in0=pool_tile[:, 0:512], scalar1=bo_sb[:, ct : ct + 1])
        nc.sync.dma_start(
            out=out[b, 128 * ct : 128 * (ct + 1), 512 * q2 : 512 * (q2 + 1)],
            in_=ob[:],
        )

    # ---------- prologue stage A ----------
    # b0's projections run as a dense PE burst at normal priority (they gate
    # the first strips and the burst warms the HAM).  b1's projections and V
    # tiles are emitted at LOW scheduler priority: the Tile scheduler slots
    # them into PE idle slivers during the early strips instead of ahead of
    # the critical score matmuls.
    from contextlib import contextmanager

    @contextmanager
    def lowprio(off):
        tc.cur_priority += off
        try:
            yield
        finally:
            tc.cur_priority -= off

    def drip_tile():
        return ps_drip.tile([128, 512], F32, tag="drip", name="drip_ps")

    pi = 0

    def prol_tile():
        nonlocal pi
        pi += 1
        if pi % 3 == 0:
            return drip_tile()
        return ps_ot.tile([128, 512], F32, tag="ot", name="ot_ps")

    # phase 1 (dense prologue): b0/quad0 q,k at full priority — they gate
    # strip 0 — then b0's V at slightly lower priority (PV consumes v[kt]
    # only from strip 2 onward).
    for nc2 in range(2):
        for w_sb, dst in ((wq_sb, qT_sb), (wk_sb, kT_sb)):
            emit_qk_group(0, 0, nc2, w_sb, dst, prol_tile())
    with lowprio(30):
        for nt in range(5):
            emit_v_group(0, nt, prol_tile())

    # Everything else is dripped INSIDE the strip loop ~10 strips before its
    # deadline (emission position = scheduler priority), one group per drip
    # strip, all through the dedicated drip bank:
    #   qk(b1,oc0) by strip 16; v(b1,nt) by strip 18+nt; qk(b0,oc1) by
    #   strip 32/40; qk(b1,oc1) by strip 48/56; stage C as the otn halves
    #   complete (quad1 norms land ~4 strips into the following qi block).
    drips = {}
    # v(b0, 5..7): v[nt] consumed by the PVden emitted at strip nt+2
    for i, s in enumerate((0, 1, 2)):
        drips[s] = (lambda nt=i + 5: emit_v_group(0, nt, drip_tile()))
    qk_sched = [
        (8, 1, 0, 0), (10, 1, 0, 0), (12, 1, 0, 1), (14, 1, 0, 1),
        (25, 0, 1, 0), (27, 0, 1, 0), (29, 0, 1, 1), (31, 0, 1, 1),
        (33, 1, 1, 0), (35, 1, 1, 0), (37, 1, 1, 1), (39, 1, 1, 1),
    ]
    qk_which = {}
    for s, b, oc, nc2 in qk_sched:
        key = (b, oc, nc2)
        w_sb, dst = ((wq_sb, qT_sb), (wk_sb, kT_sb))[qk_which.get(key, 0)]
        qk_which[key] = 1
        drips[s] = (lambda b=b, oc=oc, nc2=nc2, w_sb=w_sb, dst=dst:
                    emit_qk_group(b, oc, nc2, w_sb, dst, drip_tile()))
    for i, s in enumerate((16, 17, 18, 19, 20, 21, 22, 23)):
        drips[s] = (lambda nt=i: emit_v_group(1, nt, drip_tile()))
    for i, (ct,) in enumerate([(0,), (1,)]):
        drips[45 + i] = lambda ct=ct: stage_c_group(0, ct, 0, drip_tile())
        drips[53 + i] = lambda ct=ct: stage_c_group(0, ct, 1, drip_tile())
        drips[60 + i] = lambda ct=ct: stage_c_group(1, ct, 0, drip_tile())

    # ---------- softmax normalization (per qi block) ----------
    eb3 = eb_sb[:].rearrange("p (h w) -> p h w", w=GW)

    def _make_norm(ot_, den_, b_, quad_, qi_):
        # The den accumulator's rows 32h2..32h2+31 hold 32 identical copies
        # of head h2's denominators (M=32 col-tiled ones matmul), i.e. the
        # tile is ALREADY in row-broadcast layout.  The whole normalization
        # is two DVE ops: approx-reciprocal (51 ULP, plenty under the 2e-2
        # budget; denominators are benign positive sums) and one multiply.
        state = {}

        def part1():
            rdb = small.tile([128, 512], F32, tag="rdb", name="rdb_t")
            nc.vector.reciprocal_approx_fast(out=rdb[:], in_=den_[:])
            state["rdb"] = rdb

        def part2():
            nc.vector.tensor_mul(
                out=otn_sb[b_][quad_][:, 512 * qi_ : 512 * (qi_ + 1)],
                in0=ot_[:],
                in1=state["rdb"][:],
            )

        return [part1, part2]

    # ---------- stage B: 64 strips, lag-2 software pipeline ----------
    def emit_pvden(args):
        ot_, den_, b_, quad_, qi_, kt_, se_, first, last = args
        for h2 in range(4):
            nc.tensor.matmul(
                ot_[32 * h2 : 32 * (h2 + 1), :],
                lhsT=v_sb[b_][kt_][:, 33 * (4 * quad_ + h2) : 33 * (4 * quad_ + h2) + 32],
                rhs=se_[:, 512 * h2 : 512 * (h2 + 1)],
                start=first,
                stop=last,
                tile_position=(0, 32 * h2),
                skip_group_check=True,
            )
        for h2 in range(4):
            nc.tensor.matmul(
                den_[32 * h2 : 32 * (h2 + 1), :],
                lhsT=ones32_sb[:],
                rhs=se_[:, 512 * h2 : 512 * (h2 + 1)],
                start=first,
                stop=last,
                tile_position=(0, 32 * h2),
                skip_group_check=True,
            )
        if last:
            norm_parts.extend(_make_norm(ot_, den_, b_, quad_, qi_))

    BLOCKS = [(0, 0), (0, 1), (1, 0), (1, 1)]  # (quad, b)
    pending = deque()
    norm_parts = deque()
    block_acc = {}  # (qi,) accumulators for the current block

    for s in range(64):
        quad, b = BLOCKS[s // 16]
        qi = (s // 8) % 2
        kt = s % 8
        # norm part for a completed qi block (emitted BEFORE this strip's
        # lag-2 PVden so the den bank WAR resolves without a stall)
        if norm_parts:
            norm_parts.popleft()()
        if kt == 0:
            block_acc = (
                ps_ot.tile([128, 512], F32, tag="ot", name="ot_ps"),
                ps_den.tile([128, 512], F32, tag="den", name="den_ps"),
            )
        ot_cur, den_cur = block_acc
        st_lo = ps_lo.tile([128, 1024], F32, tag="stlo", name="stlo_ps")
        st_hi = ps_hi.tile([128, 1024], F32, tag="sthi", name="sthi_ps")
        se = stexp_pool.tile([128, 2048], BF16, tag="se", name="se_t")
        for h2 in range(4):
            dst = st_lo if h2 < 2 else st_hi
            nc.tensor.matmul(
                dst[:, 512 * (h2 % 2) : 512 * (h2 % 2 + 1)],
                lhsT=kT_sb[b][quad][32 * h2 : 32 * (h2 + 1), 128 * kt : 128 * (kt + 1)],
                rhs=qT_sb[b][quad][32 * h2 : 32 * (h2 + 1), 512 * qi : 512 * (qi + 1)],
                start=True,
                stop=True,
                tile_position=(32 * h2, 0),
            )
        nc.scalar.activation(out=se[:, 0:1024], in_=st_lo[:], func=AF.Exp)
        nc.scalar.activation(out=se[:, 1024:2048], in_=st_hi[:], func=AF.Exp)
        off = 896 - 128 * kt + 512 * qi
        nc.vector.tensor_mul(
            out=se[:].rearrange("p (h q) -> p h q", h=4),
            in0=se[:].rearrange("p (h q) -> p h q", h=4),
            in1=eb3[:, 4 * quad : 4 * quad + 4, off : off + 512],
        )
        pending.append((ot_cur, den_cur, b, quad, qi, kt, se, kt == 0, kt == 7))
        if len(pending) > 2:
            emit_pvden(pending.popleft())
        if s in drips:
            drips[s]()

    # ---------- tail ----------
    while pending:
        emit_pvden(pending.popleft())
    while norm_parts:
        norm_parts.popleft()()
    # the two final output-projection groups on separate banks so they run
    # concurrently
    stage_c_group(1, 0, 1, drip_tile())
    stage_c_group(1, 1, 1, ps_ot.tile([128, 512], F32, tag="ot", name="ot_ps"))


def build():
    nc = bacc.Bacc("TRN2", target_bir_lowering=False, debug=False, num_devices=N_CORES)
    io = {
        "x": nc.dram_tensor("x", [B_LOC, C, N], BF16, kind="ExternalInput").ap(),
        "wqT": nc.dram_tensor("wqT", [C, C], BF16, kind="ExternalInput").ap(),
        "wkT": nc.dram_tensor("wkT", [C, C], BF16, kind="ExternalInput").ap(),
        "wvT": nc.dram_tensor("wvT", [C, C], BF16, kind="ExternalInput").ap(),
        "woT": nc.dram_tensor("woT", [C, C], BF16, kind="ExternalInput").ap(),
        "bo": nc.dram_tensor("bo", [C, 1], F32, kind="ExternalInput").ap(),
        "eb": nc.dram_tensor("eb", [HEADS, 128, GW], BF16, kind="ExternalInput").ap(),
        "out": nc.dram_tensor("out", [B_LOC, C, N], BF16, kind="ExternalOutput").ap(),
    }
    with tile.TileContext(nc) as tc:
        _emit(tc, io)
    nc.compile()
    return nc


_CACHE: dict = {}


def _get_nc():
    if "nc" not in _CACHE:
        _CACHE["nc"] = build()
    return _CACHE["nc"]


def make_in_maps(x, Wq, Wk, Wv, Wo, bo, rel_bias, rel_idx=None):
    """Host-side sharding/layout prep. Returns per-core input maps."""
    import ml_dtypes

    bf16 = ml_dtypes.bfloat16
    x = np.asarray(x, np.float32)
    b, c, H, W = x.shape
    assert (b, c, H * W) == (B_LOC * N_CORES, C, N)
    xr = np.ascontiguousarray(x.reshape(b, c, N).astype(bf16))
    wqT = np.ascontiguousarray(np.asarray(Wq, np.float32).T.astype(bf16))
    wkT = np.ascontiguousarray((np.asarray(Wk, np.float32) * SCALE).T.astype(bf16))
    wvT = np.ascontiguousarray(np.asarray(Wv, np.float32).T.astype(bf16))
    woT = np.ascontiguousarray(np.asarray(Wo, np.float32).T.astype(bf16))
    bo2 = np.ascontiguousarray(np.asarray(bo, np.float32).reshape(C, 1))
    rb = np.asarray(rel_bias, np.float32)
    idx = G0 + np.arange(128)[:, None] - np.arange(GW)[None, :]
    ebmat = np.ascontiguousarray(np.exp(rb[:, idx]).astype(bf16))  # [8, 128, GW]
    shared = dict(wqT=wqT, wkT=wkT, wvT=wvT, woT=woT, bo=bo2, eb=ebmat)
    return [
        dict(x=np.ascontiguousarray(xr[B_LOC * i : B_LOC * (i + 1)]), **shared)
        for i in range(N_CORES)
    ]


def _install_ntff_hook_shim():
    """bass_utils fetches the axon NTFF hook via antenv.axon_hooks, which this
    image's antenv lacks; synthesize it from trn_agent_boot's ctypes hook."""
    import sys
    import types

    try:
        from antenv.axon_hooks import get_axon_ntff_profile_hook  # noqa: F401

        return
    except ImportError:
        pass
    hook = None
    try:
        from trn_agent_boot.trn_boot import _ntff_profile_via_ctypes

        hook = _ntff_profile_via_ctypes("/opt/axon/libaxon_pjrt.so")
    except Exception:
        pass
    mod = types.ModuleType("antenv.axon_hooks")
    state = {"hook": hook}
    mod.get_axon_ntff_profile_hook = lambda: state["hook"]
    mod.set_axon_ntff_profile_hook = lambda h: state.__setitem__("hook", h)
    sys.modules["antenv.axon_hooks"] = mod


def run(inputs: dict, trace: bool = False):
    """Run on the 8 cores; returns (full_output, BassKernelResults)."""
    if trace:
        _install_ntff_hook_shim()
    in_maps = make_in_maps(**inputs)
    nc = _get_nc()
    res = bass_utils.run_bass_kernel_spmd(
        nc, in_maps, core_ids=list(range(N_CORES)), trace=trace
    )
    outs = np.stack([np.asarray(res.results[i]["out"], np.float32) for i in range(N_CORES)])
    out = outs.reshape(B_LOC * N_CORES, C, 32, 32)
    return out, res


def kernel(**inputs) -> np.ndarray:
    out, _ = run(inputs)
    return out


# revision 22
# speedup vs baseline: 1.1376x; 1.0076x over previous
"""CoAtNet relative attention kernel for Trainium2 (Bass/Tile), 8 NeuronCores.

Problem (per full input):
  x [16, 256, 32, 32] f32; Wq/Wk/Wv [256, 256]; Wo [256, 256]; bo [256];
  rel_bias [8, 3969]; rel_idx [1024, 1024] int32 (static pattern).
  out[b] = softmax(q k^T / sqrt(d) + bias) v  projected back, heads=8, d=32.

Sharding: data-parallel over batch — each of the 8 cores handles 2 batches
with identical programs (SPMD, no collectives).

Key structural facts used:
  * rel_idx[p, q] == (q - p) + 1056 exactly (the reference's quirky *W stride
    collapses the 2D relative index to 1D Toeplitz).  So the [1024, 1024]
    bias matrix per head is bias[p, q] = rel_bias[h, q - p + 1056] and any
    [128, width] tile of it (keys on partitions) is a contiguous slice of a
    small "sheared" tile  G[h, i, j'] = rel_bias[h, 1952 + i - j']  of shape
    [128, 1920].  No gather on device at all.  The bias is applied as
    exp(S+B) = exp(S) * exp(B) with exp(B) precomputed, so the application
    is a bf16 2x-mode multiply instead of an fp32 1x add.
  * Everything is computed in "transposed" layout so no transposes are ever
    needed: x arrives as [c, n] per batch; Q^T/K^T = W @ x are [d_all, n];
    scores are built as S^T [keys, queries]; P@V uses lhsT = V directly;
    and the final projection produces out^T [c, n], exactly the output
    memory layout.
  * The kernel is ACT(exp)-throughput-bound: 16.8M exps/core at 1 elem/
    lane/cycle @1.2GHz is a ~110us floor.  The schedule is built so the
    ScalarE never waits: each strip's exp is SPLIT into two ACTIVATEs over
    separate PSUM tiles (st_lo banks 0-1, st_hi banks 2-3) so the next
    strip's score matmuls can overwrite the low banks while ACT still
    processes the high banks (the single-buffered [128,2048] fp32 score
    tile cannot be double-buffered: TRN2 matmuls write fp32-only and PSUM
    has just 8 banks).  PV/den matmuls run TWO strips behind the score
    matmuls so a DVE-gated PV never head-of-line blocks ready ST work in
    the PE FIFO.  All projection / output-projection / softmax-
    normalization work is dripped into per-strip slack slots on the other
    engines.
"""

import numpy as np
from collections import deque
from contextlib import ExitStack

import concourse.bass as bass
import concourse.bacc as bacc
import concourse.mybir as mybir
import concourse.tile as tile
from concourse import bass_utils
from concourse._compat import with_exitstack

HEADS = 8
D = 32  # head dim
C = 256  # channels = heads * D
N = 1024  # tokens = 32 * 32
B_LOC = 2  # batches per core
N_CORES = 8
SCALE = D ** -0.5
GW = 1920  # sheared bias tile width
G0 = 1952  # G[h, i, j'] = rel_bias[h, G0 + i - j']

F32 = mybir.dt.float32
BF16 = mybir.dt.bfloat16
AF = mybir.ActivationFunctionType


@with_exitstack
def _emit(ctx: ExitStack, tc: tile.TileContext, io: dict):
    nc = tc.nc
    x, wqT, wkT, wvT, woT, bo, eb, out = (
        io[k] for k in ("x", "wqT", "wkT", "wvT", "woT", "bo", "eb", "out")
    )

    persist = ctx.enter_context(tc.tile_pool(name="persist", bufs=1))
    stexp_pool = ctx.enter_context(tc.tile_pool(name="stexp", bufs=4))
    small = ctx.enter_context(tc.tile_pool(name="small", bufs=2))
    outp = ctx.enter_context(tc.tile_pool(name="outp", bufs=4))
    # PSUM budget (8 banks): st_lo 2 + st_hi 2 + ot 2x1 + den 1x1 + drip 1.
    ps_lo = ctx.enter_context(tc.tile_pool(name="ps_lo", bufs=1, space="PSUM"))
    ps_hi = ctx.enter_context(tc.tile_pool(name="ps_hi", bufs=1, space="PSUM"))
    ps_ot = ctx.enter_context(tc.tile_pool(name="ps_ot", bufs=2, space="PSUM"))
    ps_den = ctx.enter_context(tc.tile_pool(name="ps_den", bufs=1, space="PSUM"))
    ps_drip = ctx.enter_context(tc.tile_pool(name="ps_drip", bufs=1, space="PSUM"))

    # ---------- DMAs: everything in flight up front ----------
    # x[b0] + projection weights first (they gate the first score matmuls);
    # the bulky 3.75MB of exp-bias tiles follow (first needed only by the
    # first bias multiply, ~2us after the first exp).
    ones32_sb = persist.tile([128, 32], BF16, tag="ones32", name="ones32")
    nc.vector.memset(ones32_sb[:], 1.0)
    # warm up the exp table set (~2.7us ACT_TABLE_LOAD) under the prologue
    warm = small.tile([1, 32], F32, tag="warm", name="warm_t")
    nc.scalar.activation(out=warm[:], in_=ones32_sb[0:1, :], func=AF.Exp)
    x_sb = [persist.tile([128, 2 * N], BF16, tag=f"x{b}", name=f"x{b}") for b in range(B_LOC)]
    nc.sync.dma_start(
        out=x_sb[0][:].rearrange("p (cc q) -> p cc q", q=N),
        in_=x[0].rearrange("(cc p) q -> p cc q", p=128),
    )
    w_tiles = {}
    for wsrc, nm in ((wqT, "wq"), (wkT, "wk"), (wvT, "wv"), (woT, "wo")):
        t = persist.tile([128, 2 * C], BF16, tag=nm, name=nm)
        nc.sync.dma_start(
            out=t[:].rearrange("p (cc q) -> p cc q", q=C),
            in_=wsrc.rearrange("(cc p) q -> p cc q", p=128),
        )
        w_tiles[nm] = t
    wq_sb, wk_sb, wv_sb, wo_sb = (w_tiles[n] for n in ("wq", "wk", "wv", "wo"))
    bo_sb = persist.tile([128, 2], F32, tag="bo", name="bo")
    nc.sync.dma_start(
        out=bo_sb[:], in_=bo.rearrange("(cc p) one -> p (cc one)", p=128)
    )
    eb_sb = persist.tile([128, HEADS * GW], BF16, tag="eb", name="eb_sb")
    for h in range(HEADS):
        nc.sync.dma_start(out=eb_sb[:, GW * h : GW * (h + 1)], in_=eb[h])
    nc.sync.dma_start(
        out=x_sb[1][:].rearrange("p (cc q) -> p cc q", q=N),
        in_=x[1].rearrange("(cc p) q -> p cc q", p=128),
    )

    # ---------- persistent stage-A outputs ----------
    qT_sb = [[persist.tile([128, N], BF16, tag=f"qT{b}_{oc}", name=f"qT{b}_{oc}") for oc in range(2)] for b in range(B_LOC)]
    kT_sb = [[persist.tile([128, N], BF16, tag=f"kT{b}_{oc}", name=f"kT{b}_{oc}") for oc in range(2)] for b in range(B_LOC)]
    # v: [n, o] layout, 8 row tiles of 128 tokens, ones column per head
    # (33 cols/head) so P@V emits the softmax denominator via ones32 matmuls.
    v_sb = [[persist.tile([128, 33 * HEADS], BF16, tag=f"v{b}_{nt}", name=f"v{b}_{nt}") for nt in range(8)] for b in range(B_LOC)]
    otn_sb = [[persist.tile([128, N], BF16, tag=f"otn{b}_{ch}", name=f"otn{b}_{ch}") for ch in range(2)] for b in range(B_LOC)]

    def emit_qk_group(b, oc, nc2, w_sb, dst, pool_tile):
        for cc in range(2):
            nc.tensor.matmul(
                pool_tile[:, 0:512],
                lhsT=w_sb[:, 256 * cc + 128 * oc : 256 * cc + 128 * (oc + 1)],
                rhs=x_sb[b][:, 1024 * cc + 512 * nc2 : 1024 * cc + 512 * (nc2 + 1)],
                start=(cc == 0),
                stop=(cc == 1),
            )
        nc.vector.tensor_copy(
            out=dst[b][oc][:, 512 * nc2 : 512 * (nc2 + 1)], in_=pool_tile[:, 0:512]
        )

    def emit_v_group(b, nt, pool_tile):
        for cc in range(2):
            nc.tensor.matmul(
                pool_tile[:, 0:C],
                lhsT=x_sb[b][:, 1024 * cc + 128 * nt : 1024 * cc + 128 * (nt + 1)],
                rhs=wv_sb[:, 256 * cc : 256 * (cc + 1)],
                start=(cc == 0),
                stop=(cc == 1),
            )
        v33 = v_sb[b][nt][:].rearrange("p (h w) -> p h w", w=33)
        nc.vector.tensor_copy(
            out=v33[:, :, 0:32], in_=pool_tile[:, 0:C].rearrange("p (h w) -> p h w", w=32)
        )
        nc.vector.memset(v33[:, :, 32:33], 1.0)

    def stage_c_group(b, ct, q2, pool_tile):
        for ch in range(2):
            nc.tensor.matmul(
                pool_tile[:, 0:512],
                lhsT=wo_sb[:, 256 * ch + 128 * ct : 256 * ch + 128 * (ct + 1)],
                rhs=otn_sb[b][ch][:, 512 * q2 : 512 * (q2 + 1)],
                start=(ch == 0),
                stop=(ch == 1),
            )
        ob = outp.tile([128, 512], BF16, tag="ob", name="ob_t")
        nc.vector.tensor_scalar_add(out=ob[:], in0=pool_tile[:, 0:512], scalar1=bo_sb[:, ct : ct + 1])
        nc.sync.dma_start(
            out=out[b, 128 * ct : 128 * (ct + 1), 512 * q2 : 512 * (q2 + 1)],
            in_=ob[:],
        )

    # ---------- prologue stage A ----------
    # b0's projections run as a dense PE burst at normal priority (they gate
    # the first strips and the burst warms the HAM).  b1's projections and V
    # tiles are emitted at LOW scheduler priority: the Tile scheduler slots
    # them into PE idle slivers during the early strips instead of ahead of
    # the critical score matmuls.
    from contextlib import contextmanager

    @contextmanager
    def lowprio(off):
        tc.cur_priority += off
        try:
            yield
        finally:
            tc.cur_priority -= off

    def drip_tile():
        return ps_drip.tile([128, 512], F32, tag="drip", name="drip_ps")

    pi = 0

    def prol_tile():
        nonlocal pi
        pi += 1
        if pi % 3 == 0:
            return drip_tile()
        return ps_ot.tile([128, 512], F32, tag="ot", name="ot_ps")

    # phase 1 (dense prologue): b0/quad0 q,k at full priority — they gate
    # strip 0 — then b0's V at slightly lower priority (PV consumes v[kt]
    # only from strip 2 onward).
    for nc2 in range(2):
        for w_sb, dst in ((wq_sb, qT_sb), (wk_sb, kT_sb)):
            emit_qk_group(0, 0, nc2, w_sb, dst, prol_tile())
    with lowprio(30):
        for nt in range(5):
            emit_v_group(0, nt, prol_tile())

    # Everything else is dripped INSIDE the strip loop ~10 strips before its
    # deadline (emission position = scheduler priority), one group per drip
    # strip, all through the dedicated drip bank:
    #   qk(b1,oc0) by strip 16; v(b1,nt) by strip 18+nt; qk(b0,oc1) by
    #   strip 32/40; qk(b1,oc1) by strip 48/56; stage C as the otn halves
    #   complete (quad1 norms land ~4 strips into the following qi block).
    drips = {}
    # v(b0, 5..7): v[nt] consumed by the PVden emitted at strip nt+2; the
    # first strips are the coldest, so keep them drip-free
    for i, s in enumerate((3, 4, 5)):
        drips[s] = (lambda nt=i + 5: emit_v_group(0, nt, drip_tile()))
    qk_sched = [
        (8, 1, 0, 0), (10, 1, 0, 0), (12, 1, 0, 1), (14, 1, 0, 1),
        (25, 0, 1, 0), (27, 0, 1, 0), (29, 0, 1, 1), (31, 0, 1, 1),
        (33, 1, 1, 0), (35, 1, 1, 0), (37, 1, 1, 1), (39, 1, 1, 1),
    ]
    qk_which = {}
    for s, b, oc, nc2 in qk_sched:
        key = (b, oc, nc2)
        w_sb, dst = ((wq_sb, qT_sb), (wk_sb, kT_sb))[qk_which.get(key, 0)]
        qk_which[key] = 1
        drips[s] = (lambda b=b, oc=oc, nc2=nc2, w_sb=w_sb, dst=dst:
                    emit_qk_group(b, oc, nc2, w_sb, dst, drip_tile()))
    for i, s in enumerate((16, 17, 18, 19, 20, 21, 22, 23)):
        drips[s] = (lambda nt=i: emit_v_group(1, nt, drip_tile()))
    for i, (ct,) in enumerate([(0,), (1,)]):
        drips[45 + i] = lambda ct=ct: stage_c_group(0, ct, 0, drip_tile())
        drips[53 + i] = lambda ct=ct: stage_c_group(0, ct, 1, drip_tile())
        drips[60 + i] = lambda ct=ct: stage_c_group(1, ct, 0, drip_tile())

    # ---------- softmax normalization (per qi block) ----------
    eb3 = eb_sb[:].rearrange("p (h w) -> p h w", w=GW)

    def _make_norm(ot_, den_, b_, quad_, qi_):
        # The den accumulator's rows 32h2..32h2+31 hold 32 identical copies
        # of head h2's denominators (M=32 col-tiled ones matmul), i.e. the
        # tile is ALREADY in row-broadcast layout.  The whole normalization
        # is two DVE ops: approx-reciprocal (51 ULP, plenty under the 2e-2
        # budget; denominators are benign positive sums) and one multiply.
        state = {}

        def part1():
            rdb = small.tile([128, 512], F32, tag="rdb", name="rdb_t")
            nc.vector.reciprocal_approx_fast(out=rdb[:], in_=den_[:])
            state["rdb"] = rdb

        def part2():
            nc.vector.tensor_mul(
                out=otn_sb[b_][quad_][:, 512 * qi_ : 512 * (qi_ + 1)],
                in0=ot_[:],
                in1=state["rdb"][:],
            )

        return [part1, part2]

    # ---------- stage B: 64 strips, lag-2 software pipeline ----------
    def emit_pvden(args):
        ot_, den_, b_, quad_, qi_, kt_, se_, first, last = args
        for h2 in range(4):
            nc.tensor.matmul(
                ot_[32 * h2 : 32 * (h2 + 1), :],
                lhsT=v_sb[b_][kt_][:, 33 * (4 * quad_ + h2) : 33 * (4 * quad_ + h2) + 32],
                rhs=se_[:, 512 * h2 : 512 * (h2 + 1)],
                start=first,
                stop=last,
                tile_position=(0, 32 * h2),
                skip_group_check=True,
            )
        for h2 in range(4):
            nc.tensor.matmul(
                den_[32 * h2 : 32 * (h2 + 1), :],
                lhsT=ones32_sb[:],
                rhs=se_[:, 512 * h2 : 512 * (h2 + 1)],
                start=first,
                stop=last,
                tile_position=(0, 32 * h2),
                skip_group_check=True,
            )
        if last:
            norm_parts.extend(_make_norm(ot_, den_, b_, quad_, qi_))

    BLOCKS = [(0, 0), (0, 1), (1, 0), (1, 1)]  # (quad, b)
    pending = deque()
    norm_parts = deque()
    block_acc = {}  # (qi,) accumulators for the current block

    for s in range(64):
        quad, b = BLOCKS[s // 16]
        qi = (s // 8) % 2
        kt = s % 8
        # norm part for a completed qi block (emitted BEFORE this strip's
        # lag-2 PVden so the den bank WAR resolves without a stall)
        if norm_parts:
            norm_parts.popleft()()
        if kt == 0:
            block_acc = (
                ps_ot.tile([128, 512], F32, tag="ot", name="ot_ps"),
                ps_den.tile([128, 512], F32, tag="den", name="den_ps"),
            )
        ot_cur, den_cur = block_acc
        st_lo = ps_lo.tile([128, 1024], F32, tag="stlo", name="stlo_ps")
        st_hi = ps_hi.tile([128, 1024], F32, tag="sthi", name="sthi_ps")
        se = stexp_pool.tile([128, 2048], BF16, tag="se", name="se_t")
        for h2 in range(4):
            dst = st_lo if h2 < 2 else st_hi
            nc.tensor.matmul(
                dst[:, 512 * (h2 % 2) : 512 * (h2 % 2 + 1)],
                lhsT=kT_sb[b][quad][32 * h2 : 32 * (h2 + 1), 128 * kt : 128 * (kt + 1)],
                rhs=qT_sb[b][quad][32 * h2 : 32 * (h2 + 1), 512 * qi : 512 * (qi + 1)],
                start=True,
                stop=True,
                tile_position=(32 * h2, 0),
            )
        nc.scalar.activation(out=se[:, 0:1024], in_=st_lo[:], func=AF.Exp)
        nc.scalar.activation(out=se[:, 1024:2048], in_=st_hi[:], func=AF.Exp)
        off = 896 - 128 * kt + 512 * qi
        nc.vector.tensor_mul(
            out=se[:].rearrange("p (h q) -> p h q", h=4),
            in0=se[:].rearrange("p (h q) -> p h q", h=4),
            in1=eb3[:, 4 * quad : 4 * quad + 4, off : off + 512],
        )
        pending.append((ot_cur, den_cur, b, quad, qi, kt, se, kt == 0, kt == 7))
        if len(pending) > 2:
            emit_pvden(pending.popleft())
        if s in drips:
            drips[s]()

    # ---------- tail ----------
    while pending:
        emit_pvden(pending.popleft())
    while norm_parts:
        norm_parts.popleft()()
    # the two final output-projection groups on separate banks so they run
    # concurrently
    stage_c_group(1, 0, 1, drip_tile())
    stage_c_group(1, 1, 1, ps_ot.tile([128, 512], F32, tag="ot", name="ot_ps"))


def build():
    nc = bacc.Bacc("TRN2", target_bir_lowering=False, debug=False, num_devices=N_CORES)
    io = {
        "x": nc.dram_tensor("x", [B_LOC, C, N], BF16, kind="ExternalInput").ap(),
        "wqT": nc.dram_tensor("wqT", [C, C], BF16, kind="ExternalInput").ap(),
        "wkT": nc.dram_tensor("wkT", [C, C], BF16, kind="ExternalInput").ap(),
        "wvT": nc.dram_tensor("wvT", [C, C], BF16, kind="ExternalInput").ap(),
        "woT": nc.dram_tensor("woT", [C, C], BF16, kind="ExternalInput").ap(),
        "bo": nc.dram_tensor("bo", [C, 1], F32, kind="ExternalInput").ap(),
        "eb": nc.dram_tensor("eb", [HEADS, 128, GW], BF16, kind="ExternalInput").ap(),
        "out": nc.dram_tensor("out", [B_LOC, C, N], BF16, kind="ExternalOutput").ap(),
    }
    with tile.TileContext(nc) as tc:
        _emit(tc, io)
    nc.compile()
    return nc


_CACHE: dict = {}


def _get_nc():
    if "nc" not in _CACHE:
        _CACHE["nc"] = build()
    return _CACHE["nc"]


def make_in_maps(x, Wq, Wk, Wv, Wo, bo, rel_bias, rel_idx=None):
    """Host-side sharding/layout prep. Returns per-core input maps."""
    import ml_dtypes

    bf16 = ml_dtypes.bfloat16
    x = np.asarray(x, np.float32)
    b, c, H, W = x.shape
    assert (b, c, H * W) == (B_LOC * N_CORES, C, N)
    xr = np.ascontiguousarray(x.reshape(b, c, N).astype(bf16))
    wqT = np.ascontiguousarray(np.asarray(Wq, np.float32).T.astype(bf16))
    wkT = np.ascontiguousarray((np.asarray(Wk, np.float32) * SCALE).T.astype(bf16))
    wvT = np.ascontiguousarray(np.asarray(Wv, np.float32).T.astype(bf16))
    woT = np.ascontiguousarray(np.asarray(Wo, np.float32).T.astype(bf16))
    bo2 = np.ascontiguousarray(np.asarray(bo, np.float32).reshape(C, 1))
    rb = np.asarray(rel_bias, np.float32)
    idx = G0 + np.arange(128)[:, None] - np.arange(GW)[None, :]
    ebmat = np.ascontiguousarray(np.exp(rb[:, idx]).astype(bf16))  # [8, 128, GW]
    shared = dict(wqT=wqT, wkT=wkT, wvT=wvT, woT=woT, bo=bo2, eb=ebmat)
    return [
        dict(x=np.ascontiguousarray(xr[B_LOC * i : B_LOC * (i + 1)]), **shared)
        for i in range(N_CORES)
    ]


def _install_ntff_hook_shim():
    """bass_utils fetches the axon NTFF hook via antenv.axon_hooks, which this
    image's antenv lacks; synthesize it from trn_agent_boot's ctypes hook."""
    import sys
    import types

    try:
        from antenv.axon_hooks import get_axon_ntff_profile_hook  # noqa: F401

        return
    except ImportError:
        pass
    hook = None
    try:
        from trn_agent_boot.trn_boot import _ntff_profile_via_ctypes

        hook = _ntff_profile_via_ctypes("/opt/axon/libaxon_pjrt.so")
    except Exception:
        pass
    mod = types.ModuleType("antenv.axon_hooks")
    state = {"hook": hook}
    mod.get_axon_ntff_profile_hook = lambda: state["hook"]
    mod.set_axon_ntff_profile_hook = lambda h: state.__setitem__("hook", h)
    sys.modules["antenv.axon_hooks"] = mod


def run(inputs: dict, trace: bool = False):
    """Run on the 8 cores; returns (full_output, BassKernelResults)."""
    if trace:
        _install_ntff_hook_shim()
    in_maps = make_in_maps(**inputs)
    nc = _get_nc()
    res = bass_utils.run_bass_kernel_spmd(
        nc, in_maps, core_ids=list(range(N_CORES)), trace=trace
    )
    outs = np.stack([np.asarray(res.results[i]["out"], np.float32) for i in range(N_CORES)])
    out = outs.reshape(B_LOC * N_CORES, C, 32, 32)
    return out, res


def kernel(**inputs) -> np.ndarray:
    out, _ = run(inputs)
    return out
